# revision 2
# baseline (speedup 1.0000x reference)
"""MHSA block (patch-embed conv + relative-pos attention + MLP) on 8 NeuronCores.

Sharding: pure data-parallel over batch (64 images -> 8 per core), weights
replicated. Host does only layout prep (transposes/casts/rel-pos gather);
all model compute runs on-device via Bass/Tile.
"""
import numpy as np
import ml_dtypes
import concourse.bass as bass
import concourse.bacc as bacc
import concourse.tile as tile
from concourse import mybir
from concourse import bass_utils
from concourse.masks import make_identity

BF = ml_dtypes.bfloat16
B, CIN, D, HEADS, HD = 64, 384, 768, 12, 64
GS, ET, N = 16, 1, 257
BL = B // 8              # images per core
NT = BL * N              # 2056 packed tokens per core
MLP = 4 * D
CHUNKS = [(i * 128, min(128, NT - i * 128)) for i in range(17)]
COLT = [(c, min(512, NT - c)) for c in range(0, NT, 512)]
QCH = [(0, 86), (86, 86), (172, 85)]
MCH = [(0, 128), (128, 128), (256, 1)]

_CACHE = {}


def _rel_bias(rpb_table):
    coords = np.stack(np.meshgrid(np.arange(GS), np.arange(GS), indexing='ij'))
    cf = coords.reshape(2, -1)
    rel = (cf[:, :, None] - cf[:, None, :]).transpose(1, 2, 0)
    rel[:, :, 0] += GS - 1
    rel[:, :, 1] += GS - 1
    rel[:, :, 0] *= 2 * GS - 1
    idx = rel.sum(-1)
    out = np.zeros((N, N), dtype=np.int32)
    out[ET:, ET:] = idx
    bias = rpb_table[out]                    # [N, N, HEADS]
    return bias.transpose(2, 0, 1).astype(np.float32)   # [HEADS, N, N]


def _ln_pair(tc, nc, pools, xt, ts):
    """mean/rstd of xt[:ts, :768] -> (mean, rstd) [ts,1] f32 tiles."""
    st = pools.tile([128, 3, nc.vector.BN_STATS_DIM], mybir.dt.float32, tag="lnst")
    xg = xt.rearrange("p (n f) -> p n f", f=256)
    for i in range(3):
        nc.vector.bn_stats(out=st[:ts, i], in_=xg[:ts, i])
    mv = pools.tile([128, nc.vector.BN_AGGR_DIM], mybir.dt.float32, tag="lnmv")
    nc.vector.bn_aggr(out=mv[:ts], in_=st[:ts])
    eps = pools.tile([128, 1], mybir.dt.float32, tag="lneps")
    nc.vector.memset(eps, 1e-5)
    rs = pools.tile([128, 1], mybir.dt.float32, tag="lnrs")
    nc.scalar.activation(out=rs[:ts], in_=mv[:ts, 1:2],
                         func=mybir.ActivationFunctionType.Sqrt, bias=eps[:ts])
    nc.vector.reciprocal(out=rs[:ts], in_=rs[:ts])
    return mv, rs


def build():
    nc = bacc.Bacc("TRN2", target_bir_lowering=False, debug=False)
    f32, bf16 = mybir.dt.float32, mybir.dt.bfloat16
    di = lambda n, s, d: nc.dram_tensor(n, s, d, kind="ExternalInput").ap()
    x_in = di("x_in", [BL, 3, 128, 32, 32], bf16)
    convw = di("convw", [27, 128, 768], bf16)
    convb_bc = di("convb_bc", [128, 768], f32)
    peg_bc = di("peg_bc", [128, 768], f32)
    geo2 = di("geo2", [2, 128, 768], f32)
    y0row = di("y0row", [1, 768], f32)
    qkvw = di("qkvw", [6, 128, 2304], bf16)
    qkvb_t = di("qkvb_t", [128, 18], f32)
    projw = di("projw", [6, 128, 768], bf16)
    projb_bc = di("projb_bc", [128, 768], f32)
    fc1w = di("fc1w", [6, 128, MLP], bf16)
    fc1b_t = di("fc1b_t", [128, 24], f32)
    fc2w = di("fc2w", [24, 128, 768], bf16)
    fc2b_bc = di("fc2b_bc", [128, 768], f32)
    bias4 = di("bias4", [12, 86, 3, 257], f32)
    out_d = nc.dram_tensor("out_d", [NT, 768], f32, kind="ExternalOutput").ap()

    with tile.TileContext(nc) as tc:
        with tc.tile_pool(name="dram", bufs=1, space="DRAM") as dpool:
            y_d = dpool.tile([NT, 768], f32)
            hT_d = dpool.tile([6, 128, NT], bf16)
            qkT_d = dpool.tile([18, 128, NT], bf16)
            oT_d = dpool.tile([6, 128, NT], bf16)
            y2_d = dpool.tile([NT, 768], f32)
            h2T_d = dpool.tile([6, 128, NT], bf16)
            h3T_d = dpool.tile([24, 128, NT], bf16)

            # ---------------- Phase 1: conv + peLN + geo -> y_d ----------------
            with tc.tile_pool(name="cw", bufs=1) as cw, \
                 tc.tile_pool(name="cx", bufs=2) as cx, \
                 tc.tile_pool(name="cps", bufs=4, space="PSUM") as cps, \
                 tc.tile_pool(name="cy", bufs=3) as cy:
                wsb = cw.tile([128, 27, 768], bf16)
                for i in range(27):
                    nc.sync.dma_start(out=wsb[:, i], in_=convw[i])
                cbc = cw.tile([128, 768], f32)
                nc.sync.dma_start(out=cbc, in_=convb_bc)
                pgc = cw.tile([128, 768], f32)
                nc.sync.dma_start(out=pgc, in_=peg_bc)
                gsb = cw.tile([128, 2, 768], f32)
                for t in range(2):
                    nc.sync.dma_start(out=gsb[:, t], in_=geo2[t])
                y0sb = cw.tile([1, 768], f32)
                nc.sync.dma_start(out=y0sb, in_=y0row)
                for b in range(BL):
                    nc.sync.dma_start(out=y_d[b * N:b * N + 1, :], in_=y0sb)
                for b in range(BL):
                    xp = cx.tile([128, 3, 1089], bf16, tag="xpad")
                    nc.vector.memset(xp, 0.0)
                    for c in range(3):
                        dst = bass.AP(tensor=xp.tensor, offset=xp.offset + c * 1089 + 34,
                                      ap=[xp.ap[0], [33, 32], [1, 32]])
                        nc.sync.dma_start(out=dst, in_=x_in[b, c])
                    for t in range(2):
                        col = cx.tile([128, 27, 128], bf16, tag="col")
                        for kh in range(3):
                            for kw in range(3):
                                for c in range(3):
                                    idx = (kh * 3 + kw) * 3 + c
                                    src = bass.AP(
                                        tensor=xp.tensor,
                                        offset=xp.offset + c * 1089 + (16 * t + kh) * 33 + kw,
                                        ap=[xp.ap[0], [66, 8], [2, 16]])
                                    nc.vector.tensor_copy(col[:, idx], src)
                        yt = cy.tile([128, 768], f32, tag="yt")
                        for nh in range(2):
                            ps = cps.tile([128, 384], f32, tag="cpsum")
                            for i in range(27):
                                nc.tensor.matmul(ps, col[:, i], wsb[:, i, nh * 384:(nh + 1) * 384],
                                                 start=(i == 0), stop=(i == 26))
                            nc.vector.tensor_add(yt[:, nh * 384:(nh + 1) * 384], ps, cbc[:, nh * 384:(nh + 1) * 384])
                        mv, rs = _ln_pair(tc, nc, cy, yt, 128)
                        nc.vector.tensor_scalar(out=yt, in0=yt, scalar1=mv[:, 0:1], scalar2=rs,
                                                op0=mybir.AluOpType.subtract, op1=mybir.AluOpType.mult)
                        nc.vector.tensor_mul(yt, yt, pgc)
                        nc.vector.tensor_add(yt, yt, gsb[:, t])
                        r0 = b * N + 1 + t * 128
                        nc.sync.dma_start(out=y_d[r0:r0 + 128, :], in_=yt)

            # ---------------- Phase 2: LN1 + transpose -> hT_d ----------------
            with tc.tile_pool(name="l1", bufs=3) as l1, \
                 tc.tile_pool(name="l1c", bufs=1) as l1c, \
                 tc.tile_pool(name="l1p", bufs=4, space="PSUM") as l1p:
                idb = l1c.tile([128, 128], bf16)
                make_identity(nc, idb)
                for (t0, ts) in CHUNKS:
                    yt = l1.tile([128, 768], f32, tag="l1y")
                    nc.sync.dma_start(out=yt[:ts], in_=y_d[t0:t0 + ts, :])
                    mv, rs = _ln_pair(tc, nc, l1, yt, ts)
                    hb = l1.tile([128, 768], bf16, tag="l1h")
                    nc.vector.tensor_scalar(out=hb[:ts], in0=yt[:ts], scalar1=mv[:ts, 0:1], scalar2=rs[:ts],
                                            op0=mybir.AluOpType.subtract, op1=mybir.AluOpType.mult)
                    for k in range(6):
                        tp = l1p.tile([128, 128], bf16, tag="l1t")
                        nc.tensor.transpose(tp[:, :ts], hb[:ts, k * 128:(k + 1) * 128], idb[:ts, :ts])
                        st = l1.tile([128, 128], bf16, tag="l1s")
                        nc.vector.tensor_copy(st[:, :ts], tp[:, :ts])
                        nc.sync.dma_start(out=hT_d[k, :, t0:t0 + ts], in_=st[:, :ts])

            # ---------------- Phase 3: QKV -> qkT_d ----------------
            with tc.tile_pool(name="qw", bufs=1) as qw, \
                 tc.tile_pool(name="qa", bufs=3) as qa, \
                 tc.tile_pool(name="qp", bufs=8, space="PSUM") as qp:
                wq = qw.tile([128, 6, 2304], bf16)
                for k in range(6):
                    nc.sync.dma_start(out=wq[:, k], in_=qkvw[k])
                qb = qw.tile([128, 18], f32)
                nc.sync.dma_start(out=qb, in_=qkvb_t)
                for (c0, cs) in COLT:
                    ht = qa.tile([128, 6, 512], bf16, tag="qh")
                    for k in range(6):
                        nc.sync.dma_start(out=ht[:, k, :cs], in_=hT_d[k, :, c0:c0 + cs])
                    for dch in range(18):
                        ps = qp.tile([128, 512], f32, tag="qps")
                        for k in range(6):
                            nc.tensor.matmul(ps[:, :cs], wq[:, k, dch * 128:(dch + 1) * 128],
                                             ht[:, k, :cs], start=(k == 0), stop=(k == 5))
                        ev = qa.tile([128, 512], bf16, tag="qev")
                        nc.vector.tensor_scalar_add(out=ev[:, :cs], in0=ps[:, :cs], scalar1=qb[:, dch:dch + 1])
                        nc.sync.dma_start(out=qkT_d[dch, :, c0:c0 + cs], in_=ev[:, :cs])

            # ---------------- Phase 4: attention -> oT_d ----------------
            with tc.tile_pool(name="ac", bufs=1) as ac, \
                 tc.tile_pool(name="ab", bufs=2) as ab, \
                 tc.tile_pool(name="aw", bufs=3) as aw, \
                 tc.tile_pool(name="ap", bufs=2, space="PSUM") as app:
                idb = ac.tile([128, 128], bf16)
                make_identity(nc, idb)
                for h in range(12):
                    bsb = ab.tile([86, 3, 257], f32, tag="bias")
                    nc.sync.dma_start(out=bsb, in_=bias4[h])
                    for b in range(BL):
                        qt = aw.tile([64, 257], bf16, tag="qt")
                        kt = aw.tile([64, 257], bf16, tag="kt")
                        vt = aw.tile([64, 257], bf16, tag="vt")
                        po = (h % 2) * 64
                        nc.sync.dma_start(out=qt, in_=qkT_d[h // 2, po:po + 64, b * N:b * N + N])
                        nc.sync.dma_start(out=kt, in_=qkT_d[6 + h // 2, po:po + 64, b * N:b * N + N])
                        nc.sync.dma_start(out=vt, in_=qkT_d[12 + h // 2, po:po + 64, b * N:b * N + N])
                        vsb = aw.tile([128, 3, 64], bf16, tag="vsb")
                        for mi, (mo, ms) in enumerate(MCH):
                            tp = app.tile([128, 64], bf16, tag="vtp")
                            nc.tensor.transpose(tp[:ms, :], vt[:, mo:mo + ms], idb[:64, :64])
                            nc.vector.tensor_copy(vsb[:ms, mi], tp[:ms, :])
                        for (q0, qs) in QCH:
                            sp = app.tile([128, 257], f32, tag="sps")
                            nc.tensor.matmul(sp[:qs], qt[:, q0:q0 + qs], kt, start=True, stop=True)
                            sc = aw.tile([86, 257], f32, tag="sc")
                            nc.vector.tensor_add(sc[:qs], sp[:qs], bsb[:qs, QCH.index((q0, qs))])
                            pr = aw.tile([86, 257], bf16, tag="pr")
                            rsum = aw.tile([86, 1], f32, tag="rsum")
                            nc.scalar.activation(pr[:qs], sc[:qs], mybir.ActivationFunctionType.Exp,
                                                 accum_out=rsum[:qs])
                            nc.vector.reciprocal(rsum[:qs], rsum[:qs])
                            nc.vector.tensor_scalar_mul(pr[:qs], pr[:qs], rsum[:qs])
                            pT = aw.tile([128, 3, 86], bf16, tag="pT")
                            for mi, (mo, ms) in enumerate(MCH):
                                tp2 = app.tile([128, 86], bf16, tag="ptp")
                                nc.tensor.transpose(tp2[:ms, :qs], pr[:qs, mo:mo + ms], idb[:qs, :qs])
                                nc.vector.tensor_copy(pT[:ms, mi, :qs], tp2[:ms, :qs])
                            op = app.tile([64, 86], f32, tag="ops")
                            for mi, (mo, ms) in enumerate(MCH):
                                nc.tensor.matmul(op[:, :qs], vsb[:ms, mi], pT[:ms, mi, :qs],
                                                 start=(mi == 0), stop=(mi == 2))
                            oe = aw.tile([64, 86], bf16, tag="oe")
                            nc.vector.tensor_copy(oe[:, :qs], op[:, :qs])
                            nc.sync.dma_start(out=oT_d[h // 2, po:po + 64, b * N + q0:b * N + q0 + qs],
                                              in_=oe[:, :qs])

            # ---------------- Phase 5: proj + residual + LN2 + T -> y2_d, h2T_d ----------------
            with tc.tile_pool(name="pw", bufs=1) as pw, \
                 tc.tile_pool(name="pa", bufs=3) as pa, \
                 tc.tile_pool(name="pp", bufs=4, space="PSUM") as pp:
                wp = pw.tile([128, 6, 768], bf16)
                for k in range(6):
                    nc.sync.dma_start(out=wp[:, k], in_=projw[k])
                pbc = pw.tile([128, 768], f32)
                nc.sync.dma_start(out=pbc, in_=projb_bc)
                idb2 = pw.tile([128, 128], bf16)
                make_identity(nc, idb2)
                for (t0, ts) in CHUNKS:
                    ot = pa.tile([128, 6, 128], bf16, tag="pot")
                    for k in range(6):
                        nc.sync.dma_start(out=ot[:, k, :ts], in_=oT_d[k, :, t0:t0 + ts])
                    yt = pa.tile([128, 768], f32, tag="py")
                    nc.sync.dma_start(out=yt[:ts], in_=y_d[t0:t0 + ts, :])
                    y2 = pa.tile([128, 768], f32, tag="py2")
                    for nh in range(2):
                        ps = pp.tile([128, 384], f32, tag="pps")
                        for k in range(6):
                            nc.tensor.matmul(ps[:ts], ot[:, k, :ts], wp[:, k, nh * 384:(nh + 1) * 384],
                                             start=(k == 0), stop=(k == 5))
                        nc.vector.tensor_add(y2[:ts, nh * 384:(nh + 1) * 384], ps[:ts],
                                             yt[:ts, nh * 384:(nh + 1) * 384])
                    nc.vector.tensor_add(y2[:ts], y2[:ts], pbc[:ts])
                    nc.sync.dma_start(out=y2_d[t0:t0 + ts, :], in_=y2[:ts])
                    mv, rs = _ln_pair(tc, nc, pa, y2, ts)
                    hb = pa.tile([128, 768], bf16, tag="ph2")
                    nc.vector.tensor_scalar(out=hb[:ts], in0=y2[:ts], scalar1=mv[:ts, 0:1], scalar2=rs[:ts],
                                            op0=mybir.AluOpType.subtract, op1=mybir.AluOpType.mult)
                    for k in range(6):
                        tp = pp.tile([128, 128], bf16, tag="ptr")
                        nc.tensor.transpose(tp[:, :ts], hb[:ts, k * 128:(k + 1) * 128], idb2[:ts, :ts])
                        st = pa.tile([128, 128], bf16, tag="pst")
                        nc.vector.tensor_copy(st[:, :ts], tp[:, :ts])
                        nc.sync.dma_start(out=h2T_d[k, :, t0:t0 + ts], in_=st[:, :ts])

            # ---------------- Phase 6: FC1 + gelu -> h3T_d ----------------
            with tc.tile_pool(name="f1w", bufs=1) as f1w, \
                 tc.tile_pool(name="f1a", bufs=3) as f1a, \
                 tc.tile_pool(name="f1p", bufs=8, space="PSUM") as f1p:
                w1 = f1w.tile([128, 6, MLP], bf16)
                for k in range(6):
                    nc.sync.dma_start(out=w1[:, k], in_=fc1w[k])
                b1 = f1w.tile([128, 24], f32)
                nc.sync.dma_start(out=b1, in_=fc1b_t)
                for (c0, cs) in COLT:
                    ht = f1a.tile([128, 6, 512], bf16, tag="f1h")
                    for k in range(6):
                        nc.sync.dma_start(out=ht[:, k, :cs], in_=h2T_d[k, :, c0:c0 + cs])
                    for dch in range(24):
                        ps = f1p.tile([128, 512], f32, tag="f1ps")
                        for k in range(6):
                            nc.tensor.matmul(ps[:, :cs], w1[:, k, dch * 128:(dch + 1) * 128],
                                             ht[:, k, :cs], start=(k == 0), stop=(k == 5))
                        ev = f1a.tile([128, 512], bf16, tag="f1ev")
                        nc.scalar.activation(ev[:, :cs], ps[:, :cs], mybir.ActivationFunctionType.Gelu,
                                             bias=b1[:, dch:dch + 1])
                        nc.sync.dma_start(out=h3T_d[dch, :, c0:c0 + cs], in_=ev[:, :cs])

            # ---------------- Phase 7: FC2 + residual -> out ----------------
            with tc.tile_pool(name="f2w", bufs=1) as f2w, \
                 tc.tile_pool(name="f2a", bufs=3) as f2a, \
                 tc.tile_pool(name="f2p", bufs=8, space="PSUM") as f2p:
                w2 = f2w.tile([128, 24, 768], bf16)
                for k in range(24):
                    nc.sync.dma_start(out=w2[:, k], in_=fc2w[k])
                b2c = f2w.tile([128, 768], f32)
                nc.sync.dma_start(out=b2c, in_=fc2b_bc)
                for (t0, ts) in CHUNKS:
                    h3 = f2a.tile([128, 24, 128], bf16, tag="f2h")
                    for k in range(24):
                        nc.sync.dma_start(out=h3[:, k, :ts], in_=h3T_d[k, :, t0:t0 + ts])
                    y2 = f2a.tile([128, 768], f32, tag="f2y")
                    nc.sync.dma_start(out=y2[:ts], in_=y2_d[t0:t0 + ts, :])
                    ot = f2a.tile([128, 768], f32, tag="f2o")
                    for nh in range(2):
                        ps = f2p.tile([128, 384], f32, tag="f2ps")
                        for k in range(24):
                            nc.tensor.matmul(ps[:ts], h3[:, k, :ts], w2[:, k, nh * 384:(nh + 1) * 384],
                                             start=(k == 0), stop=(k == 23))
                        nc.vector.tensor_add(ot[:ts, nh * 384:(nh + 1) * 384], ps[:ts],
                                             y2[:ts, nh * 384:(nh + 1) * 384])
                    nc.vector.tensor_add(ot[:ts], ot[:ts], b2c[:ts])
                    nc.sync.dma_start(out=out_d[t0:t0 + ts, :], in_=ot[:ts])

    nc.compile()
    return nc


def kernel(x, H, W, geo_bias, extra_token, conv_w, conv_b, pe_g, pe_b,
           n1_g, n1_b, qkv_w, rpb_table, proj_w, proj_b, n2_g, n2_b,
           fc1_w, fc1_b, fc2_w, fc2_b):
    x = np.asarray(x, np.float32)
    f = lambda a: np.asarray(a, np.float32)
    geo_bias, extra_token = f(geo_bias), f(extra_token)
    conv_w, conv_b, pe_g, pe_b = f(conv_w), f(conv_b), f(pe_g), f(pe_b)
    n1_g, n1_b, qkv_w, rpb_table = f(n1_g), f(n1_b), f(qkv_w), f(rpb_table)
    proj_w, proj_b, n2_g, n2_b = f(proj_w), f(proj_b), f(n2_g), f(n2_b)
    fc1_w, fc1_b, fc2_w, fc2_b = f(fc1_w), f(fc1_b), f(fc2_w), f(fc2_b)

    if "nc" not in _CACHE:
        _CACHE["nc"] = build()
    nc = _CACHE["nc"]

    # host-side weight prep (layout only; LN scale folds are exact for g=1,b=0)
    cw = conv_w.transpose(2, 3, 1, 0).reshape(3, 3, 3, 128, 768).reshape(27, 128, 768)
    qkv_wf = qkv_w * n1_g[None, :]
    qkv_wf[:D] *= HD ** -0.5
    qkv_b = qkv_w @ n1_b
    qkv_b[:D] *= HD ** -0.5
    fc1_wf = fc1_w * n2_g[None, :]
    fc1_bf = fc1_b + fc1_w @ n2_b
    bias_full = _rel_bias(rpb_table)
    b4 = np.zeros((12, 86, 3, 257), np.float32)
    for qc, (q0, qs) in enumerate(QCH):
        b4[:, :qs, qc, :] = bias_full[:, q0:q0 + qs, :]

    common = {
        "convw": cw.astype(BF),
        "convb_bc": np.tile(conv_b[None, :], (128, 1)).astype(np.float32),
        "peg_bc": np.tile(pe_g[None, :], (128, 1)).astype(np.float32),
        "geo2": (geo_bias[0, 1:, :] + pe_b[None, :]).reshape(2, 128, 768).astype(np.float32),
        "y0row": (extra_token[0] + geo_bias[0, :1, :]).astype(np.float32),
        "qkvw": qkv_wf.T.reshape(6, 128, 2304).astype(BF),
        "qkvb_t": np.ascontiguousarray(qkv_b.reshape(18, 128).T).astype(np.float32),
        "projw": proj_w.T.reshape(6, 128, 768).astype(BF),
        "projb_bc": np.tile(proj_b[None, :], (128, 1)).astype(np.float32),
        "fc1w": fc1_wf.T.reshape(6, 128, MLP).astype(BF),
        "fc1b_t": np.ascontiguousarray(fc1_bf.reshape(24, 128).T).astype(np.float32),
        "fc2w": fc2_w.T.reshape(24, 128, 768).astype(BF),
        "fc2b_bc": np.tile(fc2_b[None, :], (128, 1)).astype(np.float32),
        "bias4": b4,
    }
    in_maps = []
    for c in range(8):
        xs = x[c * BL:(c + 1) * BL].reshape(BL, 3, 128, 32, 32).astype(BF)
        in_maps.append({"x_in": xs, **common})
    global _LAST_MAPS
    _LAST_MAPS = in_maps

    res = bass_utils.run_bass_kernel_spmd(nc, in_maps, core_ids=list(range(8)))
    out = np.concatenate([r["out_d"].reshape(BL, N, D) for r in res.results], axis=0)
    return out.astype(np.float32)



# revision 11
# speedup vs baseline: 1.4870x; 1.4870x over previous
"""MHSA block (patch-embed conv + relative-pos attention + MLP) on 8 NeuronCores.

Sharding: pure data-parallel over batch (64 images -> 8 per core), weights
replicated. v2: transpose-free attention (S^T = K^T Q with bias via identity
matmul into PSUM, exp batched over image pairs on ScalarE, PV with token-major
V carrying a ones column for row sums, softmax normalization via K=1 broadcast
matmul), LN1 fused into the conv phase, SBUF-resident intermediates.
"""
import numpy as np
import ml_dtypes
import concourse.bass as bass
import concourse.bacc as bacc
import concourse.tile as tile
from concourse import mybir
from concourse import bass_utils
from concourse.masks import make_identity

BF = ml_dtypes.bfloat16
B, CIN, D, HEADS, HD = 64, 384, 768, 12, 64
GS, ET, N = 16, 1, 257
BL = B // 8              # images per core
NT = BL * N              # 2056 packed tokens per core
MLP = 4 * D
CHUNKS = [(i * 128, min(128, NT - i * 128)) for i in range(17)]
COLT = [(c, min(512, NT - c)) for c in range(0, NT, 512)]
MC3 = [(0, 86), (86, 86), (172, 85)]   # attention key-chunks within an image

_CACHE = {}
_LAST_MAPS = None


def _rel_bias(rpb_table):
    coords = np.stack(np.meshgrid(np.arange(GS), np.arange(GS), indexing='ij'))
    cf = coords.reshape(2, -1)
    rel = (cf[:, :, None] - cf[:, None, :]).transpose(1, 2, 0)
    rel[:, :, 0] += GS - 1
    rel[:, :, 1] += GS - 1
    rel[:, :, 0] *= 2 * GS - 1
    idx = rel.sum(-1)
    out = np.zeros((N, N), dtype=np.int32)
    out[ET:, ET:] = idx
    bias = rpb_table[out]                    # [N(q), N(m), HEADS]
    return bias.transpose(2, 0, 1).astype(np.float32)   # [HEADS, q, m]


def _v_pieces(t0, ts):
    """Split global-token range [t0, t0+ts) at image / key-chunk boundaries.
    Yields (psum_row, length, image, mi, vg_row)."""
    g = t0
    out = []
    while g < t0 + ts:
        b = g // N
        m = g - N * b
        mi = 0 if m < 86 else (1 if m < 172 else 2)
        m0, ms = MC3[mi]
        end = min(t0 + ts, N * b + m0 + ms)
        out.append((g - t0, end - g, b, mi, m - m0))
        g = end
    return out


def _ln_pair(nc, pool, xt, ts):
    """mean/rstd of xt[:ts, :768] -> (mean, rstd) [ts,1] f32 tiles."""
    st = pool.tile([128, 3, nc.vector.BN_STATS_DIM], mybir.dt.float32, tag="lnst")
    xg = xt.rearrange("p (n f) -> p n f", f=256)
    for i in range(3):
        nc.vector.bn_stats(out=st[:ts, i], in_=xg[:ts, i])
    mv = pool.tile([128, nc.vector.BN_AGGR_DIM], mybir.dt.float32, tag="lnmv")
    nc.vector.bn_aggr(out=mv[:ts], in_=st[:ts])
    eps = pool.tile([128, 1], mybir.dt.float32, tag="lneps")
    nc.vector.memset(eps, 1e-5)
    rs = pool.tile([128, 1], mybir.dt.float32, tag="lnrs")
    nc.scalar.activation(out=rs[:ts], in_=mv[:ts, 1:2],
                         func=mybir.ActivationFunctionType.Sqrt, bias=eps[:ts])
    nc.vector.reciprocal(out=rs[:ts], in_=rs[:ts])
    return mv, rs


def build():
    nc = bacc.Bacc("TRN2", target_bir_lowering=False, debug=False)
    f32, bf16 = mybir.dt.float32, mybir.dt.bfloat16
    di = lambda n, s, d: nc.dram_tensor(n, s, d, kind="ExternalInput").ap()
    x_in = di("x_in", [BL, 3, 128, 32, 32], bf16)
    convw = di("convw", [27, 128, 768], bf16)
    convb_bc = di("convb_bc", [128, 768], f32)
    peg_bc = di("peg_bc", [128, 768], f32)
    geo2 = di("geo2", [2, 128, 768], f32)
    y0row = di("y0row", [1, 768], f32)
    h0bc = di("h0bc", [6, 128, 8, 1], bf16)
    qkvw = di("qkvw", [6, 128, 2304], bf16)
    qb12 = di("qb12", [128, 12], f32)
    vbias_bc = di("vbias_bc", [128, 768], f32)
    biasT = di("biasT", [3, 86, 12, 264], bf16)
    projw = di("projw", [6, 128, 768], bf16)
    projb_bc = di("projb_bc", [128, 768], f32)
    fc1w = di("fc1w", [6, 128, MLP], bf16)
    fc1b_t = di("fc1b_t", [128, 24], f32)
    fc2w = di("fc2w", [24, 128, 768], bf16)
    fc2b_bc = di("fc2b_bc", [128, 768], f32)
    out_d = nc.dram_tensor("out_d", [NT, 768], f32, kind="ExternalOutput").ap()

    EXP = mybir.ActivationFunctionType.Exp
    GELU = mybir.ActivationFunctionType.Gelu

    with tile.TileContext(nc) as tc:
        with tc.tile_pool(name="dram", bufs=1, space="DRAM") as dpool:
            y_d = dpool.tile([NT, 768], f32)
            y2_d = dpool.tile([NT, 768], f32)

            cst = tc.alloc_tile_pool(name="cst", bufs=1)
            idb = cst.tile([128, 128], bf16)
            make_identity(nc, idb)
            ones64 = cst.tile([1, 64], bf16)
            nc.vector.memset(ones64, 1.0)
            qb = cst.tile([128, 12], f32)
            nc.sync.dma_start(out=qb, in_=qb12)
            pbc = cst.tile([128, 768], f32)
            nc.sync.dma_start(out=pbc, in_=projb_bc)
            b1 = cst.tile([128, 24], f32)
            nc.sync.dma_start(out=b1, in_=fc1b_t)
            b2c = cst.tile([128, 768], f32)
            nc.sync.dma_start(out=b2c, in_=fc2b_bc)

            pOT = tc.alloc_tile_pool(name="pOT", bufs=1)
            oT = pOT.tile([128, 6, NT], bf16)

            pBt = tc.alloc_tile_pool(name="pBt", bufs=1)
            Bt = pBt.tile([86, 3, 12, 264], bf16)
            for mi in range(3):
                nc.sync.dma_start(out=Bt[:, mi], in_=biasT[mi])

            pH = tc.alloc_tile_pool(name="pH", bufs=1)
            hT = pH.tile([128, 6, NT], bf16)
            hTr = hT.rearrange("p k (b n) -> p k b n", n=N)
            for k in range(6):
                nc.sync.dma_start(out=hTr[:, k, :, 0:1], in_=h0bc[k])

            # ---------- Phase 1: conv + peLN + geo -> y_d ; LN1 + T -> hT ----------
            with tc.tile_pool(name="cw", bufs=1) as cw, \
                 tc.tile_pool(name="cx", bufs=2) as cx, \
                 tc.tile_pool(name="cps", bufs=4, space="PSUM") as cps, \
                 tc.tile_pool(name="cy", bufs=3) as cy, \
                 tc.tile_pool(name="ctp", bufs=4, space="PSUM") as ctp:
                wsb = cw.tile([128, 27, 768], bf16)
                for i in range(27):
                    nc.sync.dma_start(out=wsb[:, i], in_=convw[i])
                cbc = cw.tile([128, 768], f32)
                nc.sync.dma_start(out=cbc, in_=convb_bc)
                pgc = cw.tile([128, 768], f32)
                nc.sync.dma_start(out=pgc, in_=peg_bc)
                gsb = cw.tile([128, 2, 768], f32)
                for t in range(2):
                    nc.sync.dma_start(out=gsb[:, t], in_=geo2[t])
                y0sb = cw.tile([1, 768], f32)
                nc.sync.dma_start(out=y0sb, in_=y0row)
                for b in range(BL):
                    nc.sync.dma_start(out=y_d[b * N:b * N + 1, :], in_=y0sb)
                for b in range(BL):
                    xp = cx.tile([128, 3, 1089], bf16, tag="xpad")
                    nc.vector.memset(xp, 0.0)
                    for c in range(3):
                        dst = bass.AP(tensor=xp.tensor, offset=xp.offset + c * 1089 + 34,
                                      ap=[xp.ap[0], [33, 32], [1, 32]])
                        nc.sync.dma_start(out=dst, in_=x_in[b, c])
                    for t in range(2):
                        col = cx.tile([128, 27, 128], bf16, tag="col")
                        for kh in range(3):
                            for kw in range(3):
                                for c in range(3):
                                    idx = (kh * 3 + kw) * 3 + c
                                    src = bass.AP(
                                        tensor=xp.tensor,
                                        offset=xp.offset + c * 1089 + (16 * t + kh) * 33 + kw,
                                        ap=[xp.ap[0], [66, 8], [2, 16]])
                                    nc.vector.tensor_copy(col[:, idx], src)
                        yt = cy.tile([128, 768], f32, tag="yt")
                        for nh in range(2):
                            ps = cps.tile([128, 384], f32, tag="cpsum")
                            for i in range(27):
                                nc.tensor.matmul(ps, col[:, i], wsb[:, i, nh * 384:(nh + 1) * 384],
                                                 start=(i == 0), stop=(i == 26))
                            nc.vector.tensor_add(yt[:, nh * 384:(nh + 1) * 384], ps, cbc[:, nh * 384:(nh + 1) * 384])
                        mv, rs = _ln_pair(nc, cy, yt, 128)
                        nc.vector.tensor_scalar(out=yt, in0=yt, scalar1=mv[:, 0:1], scalar2=rs,
                                                op0=mybir.AluOpType.subtract, op1=mybir.AluOpType.mult)
                        nc.vector.tensor_mul(yt, yt, pgc)
                        nc.vector.tensor_add(yt, yt, gsb[:, t])
                        r0 = b * N + 1 + t * 128
                        nc.sync.dma_start(out=y_d[r0:r0 + 128, :], in_=yt)
                        # LN1 fused here
                        mv2, rs2 = _ln_pair(nc, cy, yt, 128)
                        hb = cy.tile([128, 768], bf16, tag="hb")
                        nc.vector.tensor_scalar(out=hb, in0=yt, scalar1=mv2[:, 0:1], scalar2=rs2,
                                                op0=mybir.AluOpType.subtract, op1=mybir.AluOpType.mult)
                        for k in range(6):
                            tp = ctp.tile([128, 128], bf16, tag="tp")
                            nc.tensor.transpose(tp, hb[:, k * 128:(k + 1) * 128], idb)
                            nc.vector.tensor_copy(hT[:, k, r0:r0 + 128], tp)

            # ---------- Phase 2: QKV (Q,K d-major; V token-major w/ ones col) ----------
            pQKV = tc.alloc_tile_pool(name="pQKV", bufs=1)
            Qs = pQKV.tile([128, 6, NT], bf16)
            Ks = pQKV.tile([128, 6, NT], bf16)
            Vg = pQKV.tile([128, 24, 12, 66], bf16)
            for ci in range(24):
                nc.vector.memset(Vg[:, ci, :, 64:65], 1.0)
            with tc.tile_pool(name="qw", bufs=1) as qw, \
                 tc.tile_pool(name="qp", bufs=4, space="PSUM") as qp:
                wq = qw.tile([128, 6, 2304], bf16)
                for k in range(6):
                    nc.sync.dma_start(out=wq[:, k], in_=qkvw[k])
                vbc = qw.tile([128, 768], f32)
                nc.sync.dma_start(out=vbc, in_=vbias_bc)
                for (c0, cs) in COLT:
                    for j in range(12):
                        ps = qp.tile([128, 512], f32, tag="qps")
                        for k in range(6):
                            nc.tensor.matmul(ps[:, :cs], wq[:, k, j * 128:(j + 1) * 128],
                                             hT[:, k, c0:c0 + cs], start=(k == 0), stop=(k == 5))
                        dstq = Qs if j < 6 else Ks
                        nc.vector.tensor_scalar_add(out=dstq[:, j % 6, c0:c0 + cs],
                                                    in0=ps[:, :cs], scalar1=qb[:, j:j + 1])
                vbr = vbc.rearrange("p (g h d) -> p g h d", g=2, d=64)
                for b in range(BL):
                    for mi, (m0, ms) in enumerate(MC3):
                        for nh in range(2):
                            ps = qp.tile([128, 384], f32, tag="vps")
                            c0 = b * N + m0
                            for k in range(6):
                                nc.tensor.matmul(ps[:ms], hT[:, k, c0:c0 + ms],
                                                 wq[:, k, 1536 + nh * 384:1536 + (nh + 1) * 384],
                                                 start=(k == 0), stop=(k == 5))
                            psr = ps.rearrange("p (h d) -> p h d", d=64)
                            nc.vector.tensor_add(
                                out=Vg[0:ms, b * 3 + mi, nh * 6:(nh + 1) * 6, 0:64],
                                in0=psr[0:ms],
                                in1=vbr[0:ms, nh])

            # ---------- Phase 3: attention -> oT ----------
            with tc.tile_pool(name="asm", bufs=8) as asm, \
                 tc.tile_pool(name="asp", bufs=2, space="PSUM") as asp, \
                 tc.tile_pool(name="aop", bufs=2, space="PSUM") as aop, \
                 tc.tile_pool(name="abp", bufs=2, space="PSUM") as abp:
                for pg in range(4):
                    b0, b1_ = 2 * pg, 2 * pg + 1
                    for hp in range(6):
                        es = {}
                        for mi, (m0, ms) in enumerate(MC3):
                            for hh in range(2):
                                h = 2 * hp + hh
                                po = 64 * hh
                                S = asp.tile([128, 2, 512], f32, tag="S")
                                for ii, b in enumerate((b0, b1_)):
                                    nc.tensor.matmul(S[:ms, ii, 0:257], idb[:ms, :ms],
                                                     Bt[:ms, mi, h, 0:257], start=True, stop=False)
                                    nc.tensor.matmul(S[:ms, ii, 0:257],
                                                     Ks[po:po + 64, hp, b * N + m0:b * N + m0 + ms],
                                                     Qs[po:po + 64, hp, b * N:b * N + N],
                                                     start=False, stop=True)
                                E = asm.tile([128, 2, 264], bf16, tag="E")
                                nc.scalar.activation(E[:ms, :, 0:257], S[:ms, :, 0:257], EXP)
                                es[(hh, mi)] = E
                        for hh in range(2):
                            h = 2 * hp + hh
                            for ii, b in enumerate((b0, b1_)):
                                O = aop.tile([65, 264], f32, tag="O")
                                for mi, (m0, ms) in enumerate(MC3):
                                    nc.tensor.matmul(O[:, 0:257], Vg[0:ms, b * 3 + mi, h, 0:65],
                                                     es[(hh, mi)][:ms, ii, 0:257],
                                                     start=(mi == 0), stop=(mi == 2))
                                rc = asm.tile([1, 264], bf16, tag="rc")
                                with nc.allow_low_precision("softmax denom recip bf16"):
                                    nc.vector.reciprocal(rc[:, 0:257], O[64:65, 0:257])
                                bc = abp.tile([64, 264], f32, tag="bc")
                                nc.tensor.matmul(bc[:, 0:257], ones64, rc[0:1, 0:257],
                                                 start=True, stop=True)
                                bcs = asm.tile([64, 264], f32, tag="bcs")
                                nc.vector.tensor_copy(bcs[:, 0:257], bc[:, 0:257])
                                nc.vector.tensor_mul(oT[64 * hh:64 * hh + 64, hp, b * N:b * N + N],
                                                     O[0:64, 0:257], bcs[:, 0:257])

            pQKV.release()
            pH.release()
            pBt.release()

            # ---------- Phase 4: proj + residual + LN2 + T -> y2_d, h2T ----------
            pH2 = tc.alloc_tile_pool(name="pH2", bufs=1)
            h2T = pH2.tile([128, 6, NT], bf16)
            with tc.tile_pool(name="pw", bufs=1) as pw, \
                 tc.tile_pool(name="pa", bufs=3) as pa, \
                 tc.tile_pool(name="pp", bufs=4, space="PSUM") as pp:
                wp = pw.tile([128, 6, 768], bf16)
                for k in range(6):
                    nc.sync.dma_start(out=wp[:, k], in_=projw[k])
                for (t0, ts) in CHUNKS:
                    yt = pa.tile([128, 768], f32, tag="py")
                    nc.sync.dma_start(out=yt[:ts], in_=y_d[t0:t0 + ts, :])
                    y2 = pa.tile([128, 768], f32, tag="py2")
                    for nh in range(2):
                        ps = pp.tile([128, 384], f32, tag="pps")
                        for k in range(6):
                            nc.tensor.matmul(ps[:ts], oT[:, k, t0:t0 + ts],
                                             wp[:, k, nh * 384:(nh + 1) * 384],
                                             start=(k == 0), stop=(k == 5))
                        nc.vector.tensor_add(y2[:ts, nh * 384:(nh + 1) * 384], ps[:ts],
                                             yt[:ts, nh * 384:(nh + 1) * 384])
                    nc.vector.tensor_add(y2[:ts], y2[:ts], pbc[:ts])
                    nc.sync.dma_start(out=y2_d[t0:t0 + ts, :], in_=y2[:ts])
                    mv, rs = _ln_pair(nc, pa, y2, ts)
                    hb = pa.tile([128, 768], bf16, tag="ph2")
                    nc.vector.tensor_scalar(out=hb[:ts], in0=y2[:ts], scalar1=mv[:ts, 0:1], scalar2=rs[:ts],
                                            op0=mybir.AluOpType.subtract, op1=mybir.AluOpType.mult)
                    for k in range(6):
                        tp = pp.tile([128, 128], bf16, tag="ptr")
                        nc.tensor.transpose(tp[:, :ts], hb[:ts, k * 128:(k + 1) * 128], idb[:ts, :ts])
                        nc.vector.tensor_copy(h2T[:, k, t0:t0 + ts], tp[:, :ts])

            # ---------- Phase 5: FC1 + gelu -> h3 ; FC2 + residual -> out ----------
            with tc.tile_pool(name="fw", bufs=1) as fw, \
                 tc.tile_pool(name="fa", bufs=1) as fa, \
                 tc.tile_pool(name="fy", bufs=3) as fy, \
                 tc.tile_pool(name="f1p", bufs=4, space="PSUM") as f1p, \
                 tc.tile_pool(name="f2p", bufs=4, space="PSUM") as f2p:
                w1 = fw.tile([128, 6, MLP], bf16)
                for k in range(6):
                    nc.sync.dma_start(out=w1[:, k], in_=fc1w[k])
                w2 = fw.tile([128, 24, 768], bf16)
                for k in range(24):
                    nc.sync.dma_start(out=w2[:, k], in_=fc2w[k])
                for (c0, cs) in COLT:
                    h3 = fa.tile([128, 24, 512], bf16, tag="h3")
                    for dch in range(24):
                        ps = f1p.tile([128, 512], f32, tag="f1ps")
                        for k in range(6):
                            nc.tensor.matmul(ps[:, :cs], w1[:, k, dch * 128:(dch + 1) * 128],
                                             h2T[:, k, c0:c0 + cs], start=(k == 0), stop=(k == 5))
                        nc.scalar.activation(h3[:, dch, :cs], ps[:, :cs], GELU,
                                             bias=b1[:, dch:dch + 1])
                    nsub = (cs + 127) // 128
                    for u in range(nsub):
                        u0 = c0 + u * 128
                        us = min(128, c0 + cs - u0)
                        y2t = fy.tile([128, 768], f32, tag="fy2")
                        nc.sync.dma_start(out=y2t[:us], in_=y2_d[u0:u0 + us, :])
                        ot = fy.tile([128, 768], f32, tag="fot")
                        for nh in range(2):
                            ps2 = f2p.tile([128, 384], f32, tag="f2ps")
                            for k in range(24):
                                nc.tensor.matmul(ps2[:us], h3[:, k, u * 128:u * 128 + us],
                                                 w2[:, k, nh * 384:(nh + 1) * 384],
                                                 start=(k == 0), stop=(k == 23))
                            nc.vector.tensor_add(ot[:us, nh * 384:(nh + 1) * 384], ps2[:us],
                                                 y2t[:us, nh * 384:(nh + 1) * 384])
                        nc.vector.tensor_add(ot[:us], ot[:us], b2c[:us])
                        nc.sync.dma_start(out=out_d[u0:u0 + us, :], in_=ot[:us])

            pH2.release()
            pOT.release()
            cst.release()

    nc.compile()
    return nc


def kernel(x, H, W, geo_bias, extra_token, conv_w, conv_b, pe_g, pe_b,
           n1_g, n1_b, qkv_w, rpb_table, proj_w, proj_b, n2_g, n2_b,
           fc1_w, fc1_b, fc2_w, fc2_b):
    x = np.asarray(x, np.float32)
    f = lambda a: np.asarray(a, np.float32)
    geo_bias, extra_token = f(geo_bias), f(extra_token)
    conv_w, conv_b, pe_g, pe_b = f(conv_w), f(conv_b), f(pe_g), f(pe_b)
    n1_g, n1_b, qkv_w, rpb_table = f(n1_g), f(n1_b), f(qkv_w), f(rpb_table)
    proj_w, proj_b, n2_g, n2_b = f(proj_w), f(proj_b), f(n2_g), f(n2_b)
    fc1_w, fc1_b, fc2_w, fc2_b = f(fc1_w), f(fc1_b), f(fc2_w), f(fc2_b)

    if "nc" not in _CACHE:
        _CACHE["nc"] = build()
    nc = _CACHE["nc"]

    # host-side weight prep (layout only; LN scale folds)
    cw = conv_w.transpose(2, 3, 1, 0).reshape(3, 3, 3, 128, 768).reshape(27, 128, 768)
    qkv_wf = qkv_w * n1_g[None, :]
    qkv_wf[:D] *= HD ** -0.5
    qkv_b = qkv_w @ n1_b
    qkv_b[:D] *= HD ** -0.5
    fc1_wf = fc1_w * n2_g[None, :]
    fc1_bf = fc1_b + fc1_w @ n2_b

    bias_full = _rel_bias(rpb_table)            # [12, q, m]
    BT = bias_full.transpose(0, 2, 1)           # [12, m, q]
    biasT = np.zeros((3, 86, 12, 264), np.float32)
    for mi, (m0, ms) in enumerate(MC3):
        biasT[mi, :ms, :, :N] = BT[:, m0:m0 + ms, :].transpose(1, 0, 2)

    y0 = (extra_token[0, 0] + geo_bias[0, 0]).astype(np.float32)   # [768]
    mu = y0.mean()
    var = ((y0 - mu) ** 2).mean()
    h0 = ((y0 - mu) / np.sqrt(var + 1e-5)).astype(np.float32)
    h0bc = np.broadcast_to(h0.reshape(6, 128)[:, :, None, None],
                           (6, 128, 8, 1)).astype(BF)

    common = {
        "convw": cw.astype(BF),
        "convb_bc": np.tile(conv_b[None, :], (128, 1)).astype(np.float32),
        "peg_bc": np.tile(pe_g[None, :], (128, 1)).astype(np.float32),
        "geo2": (geo_bias[0, 1:, :] + pe_b[None, :]).reshape(2, 128, 768).astype(np.float32),
        "y0row": y0[None, :].astype(np.float32),
        "h0bc": np.ascontiguousarray(h0bc),
        "qkvw": qkv_wf.T.reshape(6, 128, 2304).astype(BF),
        "qb12": np.ascontiguousarray(qkv_b[:1536].reshape(12, 128).T).astype(np.float32),
        "vbias_bc": np.tile(qkv_b[1536:][None, :], (128, 1)).astype(np.float32),
        "biasT": biasT.astype(BF),
        "projw": proj_w.T.reshape(6, 128, 768).astype(BF),
        "projb_bc": np.tile(proj_b[None, :], (128, 1)).astype(np.float32),
        "fc1w": fc1_wf.T.reshape(6, 128, MLP).astype(BF),
        "fc1b_t": np.ascontiguousarray(fc1_bf.reshape(24, 128).T).astype(np.float32),
        "fc2w": fc2_w.T.reshape(24, 128, 768).astype(BF),
        "fc2b_bc": np.tile(fc2_b[None, :], (128, 1)).astype(np.float32),
    }
    in_maps = []
    for c in range(8):
        xs = x[c * BL:(c + 1) * BL].reshape(BL, 3, 128, 32, 32).astype(BF)
        in_maps.append({"x_in": xs, **common})
    global _LAST_MAPS
    _LAST_MAPS = in_maps

    res = bass_utils.run_bass_kernel_spmd(nc, in_maps, core_ids=list(range(8)))
    out = np.concatenate([r["out_d"].reshape(BL, N, D) for r in res.results], axis=0)
    return out.astype(np.float32)


# revision 34
# speedup vs baseline: 1.5082x; 1.0143x over previous
"""MHSA block (patch-embed conv + relative-pos attention + MLP) on 8 NeuronCores.

Sharding: pure data-parallel over batch (64 images -> 8 per core), weights
replicated. v2: transpose-free attention (S^T = K^T Q with bias via identity
matmul into PSUM, exp batched over image pairs on ScalarE, PV with token-major
V carrying a ones column for row sums, softmax normalization via K=1 broadcast
matmul), LN1 fused into the conv phase, SBUF-resident intermediates.
"""
import numpy as np
import ml_dtypes
import concourse.bass as bass
import concourse.bacc as bacc
import concourse.tile as tile
from concourse import mybir
from concourse import bass_utils
from concourse.masks import make_identity

BF = ml_dtypes.bfloat16
B, CIN, D, HEADS, HD = 64, 384, 768, 12, 64
GS, ET, N = 16, 1, 257
BL = B // 8              # images per core
NT = BL * N              # 2056 packed tokens per core
MLP = 4 * D
CHUNKS = [(i * 128, min(128, NT - i * 128)) for i in range(17)]
COLT = [(c, min(512, NT - c)) for c in range(0, NT, 512)]
MC3 = [(0, 86), (86, 86), (172, 85)]   # attention key-chunks within an image

_CACHE = {}
_LAST_MAPS = None


def _rel_bias(rpb_table):
    coords = np.stack(np.meshgrid(np.arange(GS), np.arange(GS), indexing='ij'))
    cf = coords.reshape(2, -1)
    rel = (cf[:, :, None] - cf[:, None, :]).transpose(1, 2, 0)
    rel[:, :, 0] += GS - 1
    rel[:, :, 1] += GS - 1
    rel[:, :, 0] *= 2 * GS - 1
    idx = rel.sum(-1)
    out = np.zeros((N, N), dtype=np.int32)
    out[ET:, ET:] = idx
    bias = rpb_table[out]                    # [N(q), N(m), HEADS]
    return bias.transpose(2, 0, 1).astype(np.float32)   # [HEADS, q, m]


def _v_pieces(t0, ts):
    """Split global-token range [t0, t0+ts) at image / key-chunk boundaries.
    Yields (psum_row, length, image, mi, vg_row)."""
    g = t0
    out = []
    while g < t0 + ts:
        b = g // N
        m = g - N * b
        mi = 0 if m < 86 else (1 if m < 172 else 2)
        m0, ms = MC3[mi]
        end = min(t0 + ts, N * b + m0 + ms)
        out.append((g - t0, end - g, b, mi, m - m0))
        g = end
    return out


def _ln_pair(nc, pool, xt, ts):
    """mean/rstd of xt[:ts, :768] -> (mean, rstd) [ts,1] f32 tiles."""
    st = pool.tile([128, 3, nc.vector.BN_STATS_DIM], mybir.dt.float32, tag="lnst")
    xg = xt.rearrange("p (n f) -> p n f", f=256)
    for i in range(3):
        nc.vector.bn_stats(out=st[:ts, i], in_=xg[:ts, i])
    mv = pool.tile([128, nc.vector.BN_AGGR_DIM], mybir.dt.float32, tag="lnmv")
    nc.vector.bn_aggr(out=mv[:ts], in_=st[:ts])
    eps = pool.tile([128, 1], mybir.dt.float32, tag="lneps")
    nc.vector.memset(eps, 1e-5)
    rs = pool.tile([128, 1], mybir.dt.float32, tag="lnrs")
    nc.scalar.activation(out=rs[:ts], in_=mv[:ts, 1:2],
                         func=mybir.ActivationFunctionType.Sqrt, bias=eps[:ts])
    nc.vector.reciprocal(out=rs[:ts], in_=rs[:ts])
    return mv, rs


def build():
    nc = bacc.Bacc("TRN2", target_bir_lowering=False, debug=False)
    f32, bf16 = mybir.dt.float32, mybir.dt.bfloat16
    di = lambda n, s, d: nc.dram_tensor(n, s, d, kind="ExternalInput").ap()
    f8 = mybir.dt.float8e4
    f32r = mybir.dt.float32r
    DR = mybir.MatmulPerfMode.DoubleRow
    x_in = di("x_in", [BL, 3, 128, 32, 32], bf16)
    convw = di("convw", [27, 128, 768], bf16)
    convb_bc = di("convb_bc", [128, 768], f32)
    peg_bc = di("peg_bc", [128, 768], f32)
    geo2 = di("geo2", [2, 128, 768], f32)
    y0row = di("y0row", [1, 768], f32)
    h0bc = di("h0bc", [6, 128, 8, 1], bf16)
    qkvw = di("qkvw", [6, 128, 2304], bf16)
    qb12 = di("qb12", [128, 12], f32)
    vbias_bc = di("vbias_bc", [128, 768], f32)
    biasT = di("biasT", [3, 86, 12, 264], bf16)
    projw = di("projw", [6, 128, 768], bf16)
    projb_bc = di("projb_bc", [128, 768], f32)
    fc1w = di("fc1w", [6, 128, MLP], bf16)
    fc1b_t = di("fc1b_t", [128, 24], f32)
    fc2w = di("fc2w", [24, 128, 768], bf16)
    fc2b_bc = di("fc2b_bc", [128, 768], f32)
    out_d = nc.dram_tensor("out_d", [NT, 768], f32, kind="ExternalOutput").ap()

    EXP = mybir.ActivationFunctionType.Exp
    GELU = mybir.ActivationFunctionType.Gelu

    with tile.TileContext(nc) as tc:
        with tc.tile_pool(name="dram", bufs=1, space="DRAM") as dpool:
            y_d = dpool.tile([NT, 768], f32)
            y2_d = dpool.tile([NT, 768], f32)

            cst = tc.alloc_tile_pool(name="cst", bufs=1)
            idb = cst.tile([128, 128], bf16)
            make_identity(nc, idb)
            idb8 = cst.tile([128, 128], f8)
            make_identity(nc, idb8)
            ones64 = cst.tile([1, 64], bf16)
            nc.vector.memset(ones64, 1.0)
            invs = cst.tile([128, 1], f32)
            nc.vector.memset(invs, 1.0 / 256.0)
            qb = cst.tile([128, 12], f32)
            nc.sync.dma_start(out=qb, in_=qb12)
            pbc = cst.tile([128, 768], f32)
            nc.sync.dma_start(out=pbc, in_=projb_bc)
            b1 = cst.tile([128, 24], f32)
            nc.sync.dma_start(out=b1, in_=fc1b_t)
            b2c = cst.tile([128, 768], f32)
            nc.sync.dma_start(out=b2c, in_=fc2b_bc)

            pOT = tc.alloc_tile_pool(name="pOT", bufs=1)
            oT = pOT.tile([128, 6, NT], bf16)

            pBt = tc.alloc_tile_pool(name="pBt", bufs=1)
            Bt = pBt.tile([86, 3, 12, 264], bf16)
            for mi in range(3):
                nc.sync.dma_start(out=Bt[:, mi], in_=biasT[mi])

            pH = tc.alloc_tile_pool(name="pH", bufs=1)
            hT = pH.tile([128, 6, NT], bf16)
            hTr = hT.rearrange("p k (b n) -> p k b n", n=N)
            for k in range(6):
                nc.sync.dma_start(out=hTr[:, k, :, 0:1], in_=h0bc[k])

            # ---------- Phase 1: conv + peLN + geo -> y_d ; LN1 + T -> hT ----------
            with tc.tile_pool(name="cw", bufs=1) as cw, \
                 tc.tile_pool(name="cx", bufs=2) as cx, \
                 tc.tile_pool(name="cps", bufs=4, space="PSUM") as cps, \
                 tc.tile_pool(name="cy", bufs=3) as cy, \
                 tc.tile_pool(name="ctp", bufs=4, space="PSUM") as ctp:
                wsb = cw.tile([128, 27, 768], bf16)
                for i in range(27):
                    nc.sync.dma_start(out=wsb[:, i], in_=convw[i])
                cbc = cw.tile([128, 768], f32)
                nc.sync.dma_start(out=cbc, in_=convb_bc)
                pgc = cw.tile([128, 768], f32)
                nc.sync.dma_start(out=pgc, in_=peg_bc)
                gsb = cw.tile([128, 2, 768], f32)
                for t in range(2):
                    nc.sync.dma_start(out=gsb[:, t], in_=geo2[t])
                y0sb = cw.tile([1, 768], f32)
                nc.sync.dma_start(out=y0sb, in_=y0row)
                for b in range(BL):
                    nc.sync.dma_start(out=y_d[b * N:b * N + 1, :], in_=y0sb)
                for b in range(BL):
                    xr = cx.tile([128, 3, 1024], bf16, tag="xraw")
                    for c in range(3):
                        nc.sync.dma_start(out=xr[:, c].rearrange("p (h w) -> p h w", w=32),
                                          in_=x_in[b, c])
                    xp = cx.tile([128, 3, 1089], bf16, tag="xpad")
                    nc.vector.memset(xp, 0.0)
                    for c in range(3):
                        dst = bass.AP(tensor=xp.tensor, offset=xp.offset + c * 1089 + 34,
                                      ap=[xp.ap[0], [33, 32], [1, 32]])
                        nc.vector.tensor_copy(dst, xr[:, c].rearrange("p (h w) -> p h w", w=32))
                    for t in range(2):
                        col = cx.tile([128, 27, 128], bf16, tag="col")
                        for kh in range(3):
                            for kw in range(3):
                                for c in range(3):
                                    idx = (kh * 3 + kw) * 3 + c
                                    src = bass.AP(
                                        tensor=xp.tensor,
                                        offset=xp.offset + c * 1089 + (16 * t + kh) * 33 + kw,
                                        ap=[xp.ap[0], [66, 8], [2, 16]])
                                    nc.vector.tensor_copy(col[:, idx], src)
                        yt = cy.tile([128, 768], f32, tag="yt")
                        for nh in range(2):
                            ps = cps.tile([128, 384], f32, tag="cpsum")
                            for i in range(27):
                                nc.tensor.matmul(ps, col[:, i], wsb[:, i, nh * 384:(nh + 1) * 384],
                                                 start=(i == 0), stop=(i == 26))
                            nc.vector.tensor_add(yt[:, nh * 384:(nh + 1) * 384], ps, cbc[:, nh * 384:(nh + 1) * 384])
                        mv, rs = _ln_pair(nc, cy, yt, 128)
                        nc.vector.tensor_scalar(out=yt, in0=yt, scalar1=mv[:, 0:1], scalar2=rs,
                                                op0=mybir.AluOpType.subtract, op1=mybir.AluOpType.mult)
                        nc.vector.tensor_mul(yt, yt, pgc)
                        nc.vector.tensor_add(yt, yt, gsb[:, t])
                        r0 = b * N + 1 + t * 128
                        nc.sync.dma_start(out=y_d[r0:r0 + 128, :], in_=yt)
                        # LN1 fused here
                        mv2, rs2 = _ln_pair(nc, cy, yt, 128)
                        hb = cy.tile([128, 768], bf16, tag="hb")
                        nc.vector.tensor_scalar(out=hb, in0=yt, scalar1=mv2[:, 0:1], scalar2=rs2,
                                                op0=mybir.AluOpType.subtract, op1=mybir.AluOpType.mult)
                        for k in range(6):
                            tp = ctp.tile([128, 128], bf16, tag="tp")
                            nc.tensor.transpose(tp, hb[:, k * 128:(k + 1) * 128], idb)
                            nc.vector.tensor_copy(hT[:, k, r0:r0 + 128], tp)

            # ---------- Phase 2: QKV (Q,K d-major; V token-major w/ ones col) ----------
            pQKV = tc.alloc_tile_pool(name="pQKV", bufs=1)
            Qs = pQKV.tile([128, 6, NT], bf16)
            Ks = pQKV.tile([128, 6, NT], bf16)
            Vg = pQKV.tile([128, 24, 12, 66], bf16)
            for ci in range(24):
                nc.vector.memset(Vg[:, ci, :, 64:65], 1.0)
            with tc.tile_pool(name="qw", bufs=1) as qw, \
                 tc.tile_pool(name="qp", bufs=4, space="PSUM") as qp:
                wq = qw.tile([128, 6, 2304], bf16)
                for k in range(6):
                    nc.sync.dma_start(out=wq[:, k], in_=qkvw[k])
                vbc = qw.tile([128, 768], f32)
                nc.sync.dma_start(out=vbc, in_=vbias_bc)
                for (c0, cs) in COLT:
                    for j in range(12):
                        ps = qp.tile([128, 512], f32, tag="qps")
                        for k in range(6):
                            nc.tensor.matmul(ps[:, :cs], wq[:, k, j * 128:(j + 1) * 128],
                                             hT[:, k, c0:c0 + cs], start=(k == 0), stop=(k == 5))
                        dstq = Qs if j < 6 else Ks
                        nc.vector.tensor_scalar_add(out=dstq[:, j % 6, c0:c0 + cs],
                                                    in0=ps[:, :cs], scalar1=qb[:, j:j + 1])
                vbr = vbc.rearrange("p (g h d) -> p g h d", g=2, d=64)
                for b in range(BL):
                    for mi, (m0, ms) in enumerate(MC3):
                        for nh in range(2):
                            ps = qp.tile([128, 384], f32, tag="vps")
                            c0 = b * N + m0
                            for k in range(6):
                                nc.tensor.matmul(ps[:ms], hT[:, k, c0:c0 + ms],
                                                 wq[:, k, 1536 + nh * 384:1536 + (nh + 1) * 384],
                                                 start=(k == 0), stop=(k == 5))
                            psr = ps.rearrange("p (h d) -> p h d", d=64)
                            nc.vector.tensor_add(
                                out=Vg[0:ms, b * 3 + mi, nh * 6:(nh + 1) * 6, 0:64],
                                in0=psr[0:ms],
                                in1=vbr[0:ms, nh])

            # ---------- Phase 3: attention -> oT ----------
            with tc.tile_pool(name="asm", bufs=8) as asm, \
                 tc.tile_pool(name="asp", bufs=2, space="PSUM") as asp, \
                 tc.tile_pool(name="aop", bufs=2, space="PSUM") as aop, \
                 tc.tile_pool(name="abp", bufs=2, space="PSUM") as abp:
                for pg in range(4):
                    b0, b1_ = 2 * pg, 2 * pg + 1
                    for hp in range(6):
                        es = {}
                        for mi, (m0, ms) in enumerate(MC3):
                            for hh in range(2):
                                h = 2 * hp + hh
                                po = 64 * hh
                                S = asp.tile([128, 2, 512], f32, tag="S")
                                for ii, b in enumerate((b0, b1_)):
                                    nc.tensor.matmul(S[:ms, ii, 0:257], idb[:ms, :ms],
                                                     Bt[:ms, mi, h, 0:257], start=True, stop=False)
                                    nc.tensor.matmul(S[:ms, ii, 0:257],
                                                     Ks[po:po + 64, hp, b * N + m0:b * N + m0 + ms],
                                                     Qs[po:po + 64, hp, b * N:b * N + N],
                                                     start=False, stop=True)
                                E = asm.tile([128, 2, 264], bf16, tag="E")
                                nc.scalar.activation(E[:ms, :, 0:257], S[:ms, :, 0:257], EXP)
                                es[(hh, mi)] = E
                        for hh in range(2):
                            h = 2 * hp + hh
                            for ii, b in enumerate((b0, b1_)):
                                O = aop.tile([65, 264], f32, tag="O")
                                for mi, (m0, ms) in enumerate(MC3):
                                    nc.tensor.matmul(O[:, 0:257], Vg[0:ms, b * 3 + mi, h, 0:65],
                                                     es[(hh, mi)][:ms, ii, 0:257],
                                                     start=(mi == 0), stop=(mi == 2))
                                rc = asm.tile([1, 264], bf16, tag="rc")
                                with nc.allow_low_precision("softmax denom recip bf16"):
                                    nc.vector.reciprocal(rc[:, 0:257], O[64:65, 0:257])
                                bc = abp.tile([64, 264], f32, tag="bc")
                                nc.tensor.matmul(bc[:, 0:257], ones64, rc[0:1, 0:257],
                                                 start=True, stop=True)
                                bcs = asm.tile([64, 264], f32, tag="bcs")
                                nc.vector.tensor_copy(bcs[:, 0:257], bc[:, 0:257])
                                nc.vector.tensor_mul(oT[64 * hh:64 * hh + 64, hp, b * N:b * N + N],
                                                     O[0:64, 0:257], bcs[:, 0:257])

            pQKV.release()
            pH.release()
            pBt.release()

            # ---------- Phase 4: proj + residual + LN2 + T -> y2_d, h2T ----------
            pH2 = tc.alloc_tile_pool(name="pH2", bufs=1)
            h2T = pH2.tile([128, 6, NT], bf16)
            with tc.tile_pool(name="pw", bufs=1) as pw, \
                 tc.tile_pool(name="pa", bufs=3) as pa, \
                 tc.tile_pool(name="pp", bufs=4, space="PSUM") as pp:
                wp = pw.tile([128, 6, 768], bf16)
                for k in range(6):
                    nc.sync.dma_start(out=wp[:, k], in_=projw[k])
                for (t0, ts) in CHUNKS:
                    yt = pa.tile([128, 768], f32, tag="py")
                    nc.sync.dma_start(out=yt[:ts], in_=y_d[t0:t0 + ts, :])
                    y2 = pa.tile([128, 768], f32, tag="py2")
                    for nh in range(2):
                        ps = pp.tile([128, 384], f32, tag="pps")
                        for k in range(6):
                            nc.tensor.matmul(ps[:ts], oT[:, k, t0:t0 + ts],
                                             wp[:, k, nh * 384:(nh + 1) * 384],
                                             start=(k == 0), stop=(k == 5))
                        nc.vector.tensor_add(y2[:ts, nh * 384:(nh + 1) * 384], ps[:ts],
                                             yt[:ts, nh * 384:(nh + 1) * 384])
                    nc.vector.tensor_add(y2[:ts], y2[:ts], pbc[:ts])
                    nc.sync.dma_start(out=y2_d[t0:t0 + ts, :], in_=y2[:ts])
                    mv, rs = _ln_pair(nc, pa, y2, ts)
                    hb = pa.tile([128, 768], bf16, tag="ph2")
                    nc.vector.tensor_scalar(out=hb[:ts], in0=y2[:ts], scalar1=mv[:ts, 0:1], scalar2=rs[:ts],
                                            op0=mybir.AluOpType.subtract, op1=mybir.AluOpType.mult)
                    for k in range(6):
                        tp = pp.tile([128, 128], bf16, tag="ptr")
                        nc.tensor.transpose(tp[:, :ts], hb[:ts, k * 128:(k + 1) * 128], idb[:ts, :ts])
                        nc.vector.tensor_copy(h2T[:, k, t0:t0 + ts], tp[:, :ts])

            # ---------- Phase 5: FC1 + gelu -> h3 ; FC2 + residual -> out ----------
            with tc.tile_pool(name="fw", bufs=1) as fw, \
                 tc.tile_pool(name="fa", bufs=1) as fa, \
                 tc.tile_pool(name="fy", bufs=3) as fy, \
                 tc.tile_pool(name="f1p", bufs=4, space="PSUM") as f1p, \
                 tc.tile_pool(name="f2p", bufs=4, space="PSUM") as f2p:
                w1 = fw.tile([128, 6, MLP], bf16)
                for k in range(6):
                    nc.sync.dma_start(out=w1[:, k], in_=fc1w[k])
                w2 = fw.tile([128, 24, 768], bf16)
                for k in range(24):
                    nc.sync.dma_start(out=w2[:, k], in_=fc2w[k])
                for (c0, cs) in COLT:
                    h3 = fa.tile([128, 24, 512], bf16, tag="h3")
                    for dch in range(24):
                        ps = f1p.tile([128, 512], f32, tag="f1ps")
                        for k in range(6):
                            nc.tensor.matmul(ps[:, :cs], w1[:, k, dch * 128:(dch + 1) * 128],
                                             h2T[:, k, c0:c0 + cs], start=(k == 0), stop=(k == 5))
                        nc.scalar.activation(h3[:, dch, :cs], ps[:, :cs], GELU,
                                             bias=b1[:, dch:dch + 1])
                    nsub = (cs + 127) // 128
                    for u in range(nsub):
                        u0 = c0 + u * 128
                        us = min(128, c0 + cs - u0)
                        y2t = fy.tile([128, 768], f32, tag="fy2")
                        nc.sync.dma_start(out=y2t[:us], in_=y2_d[u0:u0 + us, :])
                        ot = fy.tile([128, 768], f32, tag="fot")
                        for nh in range(2):
                            ps2 = f2p.tile([128, 384], f32, tag="f2ps")
                            for k in range(24):
                                nc.tensor.matmul(ps2[:us], h3[:, k, u * 128:u * 128 + us],
                                                 w2[:, k, nh * 384:(nh + 1) * 384],
                                                 start=(k == 0), stop=(k == 23))
                            nc.vector.tensor_add(ot[:us, nh * 384:(nh + 1) * 384], ps2[:us],
                                                 y2t[:us, nh * 384:(nh + 1) * 384])
                        nc.vector.tensor_add(ot[:us], ot[:us], b2c[:us])
                        nc.sync.dma_start(out=out_d[u0:u0 + us, :], in_=ot[:us])

            pH2.release()
            pOT.release()
            cst.release()

    nc.compile()
    return nc


def kernel(x, H, W, geo_bias, extra_token, conv_w, conv_b, pe_g, pe_b,
           n1_g, n1_b, qkv_w, rpb_table, proj_w, proj_b, n2_g, n2_b,
           fc1_w, fc1_b, fc2_w, fc2_b):
    x = np.asarray(x, np.float32)
    f = lambda a: np.asarray(a, np.float32)
    geo_bias, extra_token = f(geo_bias), f(extra_token)
    conv_w, conv_b, pe_g, pe_b = f(conv_w), f(conv_b), f(pe_g), f(pe_b)
    n1_g, n1_b, qkv_w, rpb_table = f(n1_g), f(n1_b), f(qkv_w), f(rpb_table)
    proj_w, proj_b, n2_g, n2_b = f(proj_w), f(proj_b), f(n2_g), f(n2_b)
    fc1_w, fc1_b, fc2_w, fc2_b = f(fc1_w), f(fc1_b), f(fc2_w), f(fc2_b)

    if "nc" not in _CACHE:
        _CACHE["nc"] = build()
    nc = _CACHE["nc"]

    # host-side weight prep (layout only; LN scale folds)
    cw = conv_w.transpose(2, 3, 1, 0).reshape(3, 3, 3, 128, 768).reshape(27, 128, 768)
    qkv_wf = qkv_w * n1_g[None, :]
    qkv_wf[:D] *= HD ** -0.5
    qkv_b = qkv_w @ n1_b
    qkv_b[:D] *= HD ** -0.5
    fc1_wf = fc1_w * n2_g[None, :]
    fc1_bf = fc1_b + fc1_w @ n2_b

    bias_full = _rel_bias(rpb_table)            # [12, q, m]
    BT = bias_full.transpose(0, 2, 1)           # [12, m, q]
    biasT = np.zeros((3, 86, 12, 264), np.float32)
    for mi, (m0, ms) in enumerate(MC3):
        biasT[mi, :ms, :, :N] = BT[:, m0:m0 + ms, :].transpose(1, 0, 2)

    y0 = (extra_token[0, 0] + geo_bias[0, 0]).astype(np.float32)   # [768]
    mu = y0.mean()
    var = ((y0 - mu) ** 2).mean()
    h0 = ((y0 - mu) / np.sqrt(var + 1e-5)).astype(np.float32)
    h0bc = np.broadcast_to(h0.reshape(6, 128)[:, :, None, None],
                           (6, 128, 8, 1)).astype(BF)

    common = {
        "convw": cw.astype(BF),
        "convb_bc": np.tile(conv_b[None, :], (128, 1)).astype(np.float32),
        "peg_bc": np.tile(pe_g[None, :], (128, 1)).astype(np.float32),
        "geo2": (geo_bias[0, 1:, :] + pe_b[None, :]).reshape(2, 128, 768).astype(np.float32),
        "y0row": y0[None, :].astype(np.float32),
        "h0bc": np.ascontiguousarray(h0bc),
        "qkvw": qkv_wf.T.reshape(6, 128, 2304).astype(BF),
        "qb12": np.ascontiguousarray(qkv_b[:1536].reshape(12, 128).T).astype(np.float32),
        "vbias_bc": np.tile(qkv_b[1536:][None, :], (128, 1)).astype(np.float32),
        "biasT": biasT.astype(BF),
        "projw": proj_w.T.reshape(6, 128, 768).astype(BF),
        "projb_bc": np.tile(proj_b[None, :], (128, 1)).astype(np.float32),
        "fc1w": fc1_wf.T.reshape(6, 128, MLP).astype(BF),
        "fc1b_t": np.ascontiguousarray(fc1_bf.reshape(24, 128).T).astype(np.float32),
        "fc2w": fc2_w.T.reshape(24, 128, 2, 768) if False else fc2_w.T.reshape(24, 128, 768).astype(BF),
        "fc2b_bc": np.tile(fc2_b[None, :], (128, 1)).astype(np.float32),
    }
    in_maps = []
    for c in range(8):
        xs = x[c * BL:(c + 1) * BL].reshape(BL, 3, 128, 32, 32).astype(BF)
        in_maps.append({"x_in": xs, **common})
    global _LAST_MAPS
    _LAST_MAPS = in_maps

    res = bass_utils.run_bass_kernel_spmd(nc, in_maps, core_ids=list(range(8)))
    out = np.concatenate([r["out_d"].reshape(BL, N, D) for r in res.results], axis=0)
    return out.astype(np.float32)
